# revision 33
# baseline (speedup 1.0000x reference)
"""GatedGCN Trainium2 kernel — 8-core SPMD, self-contained.

Strategy (v3)
-------------
dst-shard the graph across 8 NeuronCores. Node features live in DRAM as an
fp16 table with 4 nodes packed per 256B row in FEATURE-MAJOR order
([32f x 4j] per row), so the gathered tile is contiguous in f for the DVE
multiply and the slot-reduction runs as a single XY tensor_reduce per panel
at 16-bit 2x/4x rate. Nodes are relabeled per shard by descending in-degree
and grouped into 128-node panels; each edge occupies a (node, slot) cell, so
the weighted segment-sum is a regular DVE multiply+reduce over the panel (no
scatter). Per-slot weights are expanded into 4 j-planes (weight on the src%4
lane, 0 elsewhere) which also performs the 4-way sub-row selection. The
gather stream (SWDGE, 4 queues, 1024-idx calls) is decoupled from consumers
via per-panel tiles so descriptor generation pipelines at ~2.6ns/idx. Layer
hand-off between cores is a single fp16 AllGather. All matmul operands are
fp16 (PSUM accum f32); GRU r/z gate pre-activations are summed in PSUM by
accumulating the Wih and Whh matmuls into one bank. h stays in SBUF
feature-major fp16 between layers (no DRAM roundtrip).
"""
import sys

sys.path.insert(0, "/opt/trn_rl_repo")

import numpy as np

import concourse.bacc as bacc
import concourse.bass as bass
import concourse.mybir as mybir
import concourse.tile as tile
from concourse.bass_utils import run_bass_kernel_spmd
from concourse.masks import make_identity

N = 100000
E = 1600000
H = 32
NCLS = 2
LAYERS = 2
NCORES = 8
KCH = 8  # idxs per dma_gather call = 128*KCH (SWDGE ring cap ~65-72 descs/dma)

F32 = mybir.dt.float32
F16 = mybir.dt.float16
I16 = mybir.dt.int16
AF = mybir.ActivationFunctionType
ALU = mybir.AluOpType


def _split_multiwaits(nc, max_waits=1):
    """This walrus build rejects >1 sync-wait per instruction; split extras
    onto same-engine InstNoOp predecessors (semantically identical). Also
    collectives only carry one sync update: keep the cc_sem completion update
    on the instruction and move Tile's engine-clock updates to a same-engine
    NoOp successor (fires at issue, which is what the clock tracks)."""
    ctr = 0
    for fn in nc.m.functions:
        for bb in fn.blocks:
            new_insts = []
            for inst in bb.instructions:
                si = inst.sync_info
                if (
                    isinstance(inst, mybir.InstCollectiveCompute)
                    and si is not None
                    and si.on_update
                    and len(si.on_update) > 1
                ):
                    ups = list(si.on_update)
                    keep = [u for u in ups if "cc_sem" in str(u)]
                    move = [u for u in ups if "cc_sem" not in str(u)]
                    if not keep:
                        keep, move = ups[:1], ups[1:]
                    inst.sync_info = mybir.SyncInfo(
                        on_wait=list(si.on_wait) if si.on_wait else [],
                        on_update=keep[:1],
                    )
                    ctr += 1
                    nop = mybir.InstNoOp(name=f"USPLIT-{ctr}", engine=inst.engine)
                    nop.sync_info = mybir.SyncInfo(
                        on_wait=[], on_update=move + keep[1:]
                    )
                    nc.register_instruction(nop, overwrite=True)
                    new_insts.append(inst)
                    new_insts.append(nop)
                    continue
                waits = list(si.on_wait) if si is not None and si.on_wait else []
                if len(waits) > max_waits:
                    head, tail = waits[:-max_waits], waits[-max_waits:]
                    for i in range(0, len(head), max_waits):
                        ctr += 1
                        nop = mybir.InstNoOp(name=f"WSPLIT-{ctr}", engine=inst.engine)
                        nop.sync_info = mybir.SyncInfo(
                            on_wait=head[i : i + max_waits], on_update=[]
                        )
                        nc.register_instruction(nop, overwrite=True)
                        new_insts.append(nop)
                    inst.sync_info = mybir.SyncInfo(
                        on_wait=tail,
                        on_update=list(si.on_update) if si.on_update else [],
                    )
                new_insts.append(inst)
            bb.instructions[:] = new_insts


def _seg_batches(batches):
    """Collective segment sizes (in batches): large head, small tail so the
    final boundary chain is short."""
    return [batches // 2 + 1, batches - batches // 2 - 1]


def _sizes(n):
    shard = n // NCORES
    shard_pad = -(-shard // 512) * 512
    panels = shard_pad // 128
    tabrows = NCORES * shard_pad // 4
    return shard, shard_pad, panels, tabrows


def _preprocess(edge_index, edge_weight):
    shard, shard_pad, panels, tabrows = _sizes(N)
    src_ = np.asarray(edge_index[0], dtype=np.int64)
    dst = np.asarray(edge_index[1], dtype=np.int64)
    src = src_
    w = np.asarray(edge_weight, dtype=np.float32)

    deg = np.bincount(dst, minlength=N)
    shards = np.arange(N) // shard
    order = np.lexsort((np.arange(N), -deg, shards))  # old ids by (shard, -deg)
    new_of_old = np.empty(N, dtype=np.int64)
    pos = np.arange(N)
    c_of_pos = pos // shard
    r_of_pos = pos - c_of_pos * shard
    new_of_old[order] = c_of_pos * shard_pad + r_of_pos

    s_new = new_of_old[src]
    d_new = new_of_old[dst]
    core = d_new // shard_pad
    r = d_new % shard_pad
    # slot index k per edge: occurrence number among edges sharing the dst
    eorder = np.argsort(d_new, kind="stable")
    ds = d_new[eorder]
    starts = np.r_[0, np.nonzero(np.diff(ds))[0] + 1]
    counts = np.diff(np.r_[starts, len(ds)])
    k_sorted = np.arange(len(ds)) - np.repeat(starts, counts)
    k = np.empty(src_.size, dtype=np.int64)
    k[eorder] = k_sorted

    # per-core per-panel K, unified across cores (SPMD: one program)
    deg_new = np.zeros(NCORES * shard_pad, dtype=np.int64)
    deg_new[new_of_old] = deg
    K_uni = np.zeros(panels, dtype=np.int64)
    for c in range(NCORES):
        base = c * shard_pad
        firsts = deg_new[base : base + shard_pad : 128]  # max of each panel
        K_uni = np.maximum(K_uni, firsts)
    K_uni = K_uni.astype(np.int64)
    sumK = int(K_uni.sum())
    col0 = np.zeros(panels + 1, dtype=np.int64)
    col0[1:] = np.cumsum(128 * K_uni)
    slots_total = int(col0[-1])

    p_of_edge = r // 128
    q_of_edge = r % 128
    slotpos = col0[p_of_edge] + k * 128 + q_of_edge

    # segment-major table rows: the AllGather runs in segments, so the table
    # is [8 x seg0 rows][8 x seg1 rows]... with per-core blocks contiguous
    seg_rows = [b * 128 for b in _seg_batches(shard_pad // 512)]
    s_core = s_new // shard_pad
    s_row = (s_new % shard_pad) >> 2
    row_glob = np.zeros_like(s_row)
    r0 = 0
    t0 = 0
    for sr in seg_rows:
        m2 = (s_row >= r0) & (s_row < r0 + sr)
        row_glob[m2] = t0 + s_core[m2] * sr + (s_row[m2] - r0)
        r0 += sr
        t0 += NCORES * sr
    row_glob = row_glob.astype(np.int16)
    idx_imgs = np.zeros((NCORES, 128, 8 * sumK), dtype=np.int16)
    w4_imgs = np.zeros((NCORES, 128, 4 * sumK), dtype=np.float16)
    for c in range(NCORES):
        m = core == c
        ia = np.zeros(slots_total, dtype=np.int16)
        wa = np.zeros(slots_total * 4, dtype=np.float16)
        ia[slotpos[m]] = row_glob[m]
        wa[slotpos[m] * 4 + (s_new[m] & 3)] = w[m].astype(np.float16)
        icols = 0
        wcols = 0
        for p in range(panels):
            K = int(K_uni[p])
            if K == 0:
                continue
            a, b = int(col0[p]), int(col0[p + 1])
            blk = ia[a:b].reshape(K * 8, 16).T  # [16, 8K]
            idx_imgs[c, :, icols : icols + 8 * K] = np.tile(blk, (8, 1))
            wb = wa[4 * a : 4 * b].reshape(K, 128, 4)
            w4_imgs[c, :, wcols : wcols + 4 * K] = wb.transpose(1, 0, 2).reshape(
                128, 4 * K
            )
            icols += 8 * K
            wcols += 4 * K
    return {
        "order": order,
        "K_uni": K_uni,
        "idx_imgs": idx_imgs,
        "w4_imgs": w4_imgs,
        "sumK": sumK,
    }


def _w33(WT, b):
    """[K_in, K_out] weight^T stacked with bias row -> [K_in+1, K_out] f16."""
    return np.concatenate(
        [np.asarray(WT, np.float32), np.asarray(b, np.float32)[None, :]], axis=0
    ).astype(np.float16)


_BUILD_CACHE = {}


def _build(K_uni, fuse):
    key = (tuple(int(x) for x in K_uni), tuple(float(x) for x in fuse))
    if key in _BUILD_CACHE:
        return _BUILD_CACHE[key]

    shard, shard_pad, panels, tabrows = _sizes(N)
    sumK = int(np.sum(K_uni))
    batches = panels // 4

    nc = bacc.Bacc(
        "TRN2",
        target_bir_lowering=False,
        debug=False,
        num_devices=NCORES,
        num_swdge_queues=4,
        dynamic_dma_scratch_size=8192,
    )
    lp_cm = nc.allow_low_precision(reason="f16 msg accumulate, tol 2e-2")
    lp_cm.__enter__()
    xT = nc.dram_tensor("xT", [H + 1, shard_pad], F16, kind="ExternalInput").ap()
    idx_d = nc.dram_tensor("idx", [128, 8 * sumK], I16, kind="ExternalInput").ap()
    w4_d = nc.dram_tensor("w4", [128, 4 * sumK], F16, kind="ExternalInput").ap()
    w1_d = nc.dram_tensor("w1", [H + 1, H], F16, kind="ExternalInput").ap()
    wnn_d = nc.dram_tensor("wnn", [LAYERS * (H + 1), H], F16, kind="ExternalInput").ap()
    wih_d = nc.dram_tensor("wih", [H + 1, 3 * H], F16, kind="ExternalInput").ap()
    whh_d = nc.dram_tensor("whh", [H + 1, 3 * H], F16, kind="ExternalInput").ap()
    wout_d = nc.dram_tensor("wout", [H + 1, NCLS], F16, kind="ExternalInput").ap()
    bv_d = nc.dram_tensor("bv", [2 * H, 8], F32, kind="ExternalInput").ap()
    out_d = nc.dram_tensor("out", [128, NCLS * panels], F32, kind="ExternalOutput").ap()

    shard_buf = nc.dram_tensor("shard_buf", [shard_pad // 4, 128], F16).ap()
    tables = [
        nc.dram_tensor(f"table{i}", [tabrows, 128], F16, addr_space="Shared").ap()
        for i in range(LAYERS)
    ]

    # persistent SBUF: gather indices / weight planes / feature-major h (f16,
    # [33, shard_pad] with a ones row at partition 32 for bias matmuls)
    idx_sb = nc.alloc_sbuf_tensor("idx_sb", [128, 8 * sumK], I16).ap()
    bv_sb = nc.alloc_sbuf_tensor("bv_sb", [2 * H, 8], F32).ap()
    w4_sb = nc.alloc_sbuf_tensor("w4_sb", [128, 4 * sumK], F16).ap()
    xf = [
        nc.alloc_sbuf_tensor(f"xf{i}", [H, shard_pad], F16).ap() for i in range(2)
    ]

    cc_sem_cm = nc.semaphore("cc_sem")
    cc_sem = cc_sem_cm.__enter__()

    call_q = [0]
    SEG_B = _seg_batches(batches)          # batches per collective segment
    SEG_R = [b * 128 for b in SEG_B]       # shard_buf rows per segment
    NSEG = len(SEG_B)
    rg = [list(range(NCORES))]

    def emit_seg_coll(li, seg):
        """AllGather one shard_buf segment into the segment-major table."""
        tab_flat = tables[li].rearrange("a b -> (a b)")
        r0 = sum(SEG_R[:seg])
        r1 = r0 + SEG_R[seg]
        t0 = NCORES * r0 * 128
        t1 = NCORES * r1 * 128
        nc.gpsimd.collective_compute(
            "AllGather", ALU.bypass, replica_groups=rg,
            ins=[shard_buf[r0:r1, :].rearrange("a b -> (a b)")],
            outs=[tab_flat[t0:t1]],
        ).then_inc(cc_sem, 1)

    def build_rows(tc, pools, src33, b):
        """Emit table rows ([32f x 4j] per 4-node row) for batch b from the
        feature-major [33, shard_pad] f16 tensor src33, then DMA to shard_buf."""
        cst, pj, sp = pools
        cols = slice(512 * b, 512 * (b + 1))
        jt = pj.tile([128, 4, H], F16)
        for j in range(4):
            nc.tensor.transpose(
                out=jt[:, j, :],
                in_=src33[:, cols][:, j::4],
                identity=cst["ident32h"][:],
            )
        hfp = sp.tile([128, H, 4], F16)
        # reorder (j, f) -> (f, j): iterate out in (j, f) order to match jt
        nc.vector.tensor_copy(
            out=hfp[:].rearrange("p f j -> p j f"),
            in_=jt[:],
        )
        nc.sync.dma_start(
            out=shard_buf[128 * b : 128 * (b + 1), :],
            in_=hfp[:].rearrange("p a b -> p (a b)"),
        )

    # ---------------- TC1: h1 = relu(x @ W1 + b1) for own shard ----------
    with tile.TileContext(nc) as tc:
        with (
            tc.tile_pool(name="cp", bufs=2) as cp,
            tc.tile_pool(name="sp", bufs=2) as sp,
            tc.tile_pool(name="const1", bufs=1) as cst_p,
            tc.tile_pool(name="pp", bufs=2, space="PSUM") as pp,
            tc.tile_pool(name="pj", bufs=2, space="PSUM") as pj,
        ):
            cst = {}
            ident32h = cst_p.tile([H, H], F16)
            cst["ident32h"] = ident32h
            make_identity(nc, cst["ident32h"][:])
            w1t = cst_p.tile([H + 1, H], F16)
            nc.sync.dma_start(out=w1t[:], in_=w1_d[:])
            nc.scalar.dma_start(out=idx_sb[:], in_=idx_d[:])
            nc.sync.dma_start(out=bv_sb[:], in_=bv_d[:])
            nc.scalar.dma_start(out=w4_sb[:], in_=w4_d[:])
            for b in range(batches):
                cols = slice(512 * b, 512 * (b + 1))
                xt = cp.tile([H + 1, 512], F16)
                nc.sync.dma_start(out=xt[:], in_=xT[:, cols])
                ps = pp.tile([H, 512], F32)
                nc.tensor.matmul(out=ps[:], lhsT=w1t[:], rhs=xt[:], start=True, stop=True)
                nc.scalar.activation(xf[0][:, cols], ps[:], AF.Relu)
                build_rows(tc, (cst, pj, sp), xf[0], b)
                for sg in range(NSEG - 1):
                    if b == sum(SEG_B[: sg + 1]) - 1:
                        emit_seg_coll(0, sg)
            emit_seg_coll(0, NSEG - 1)

    nc.gpsimd.wait_ge(cc_sem, NSEG)

    # ---------------- layers ----------------
    def build_layer(li, b0, b1):
        last = li == LAYERS - 1
        col0i = np.zeros(panels + 1, dtype=np.int64)
        col0i[1:] = np.cumsum(8 * K_uni)
        col0w = np.zeros(panels + 1, dtype=np.int64)
        col0w[1:] = np.cumsum(4 * K_uni)
        xf_cur, xf_nxt = xf[li % 2], xf[(li + 1) % 2]
        if not last:
            # build_rows below overwrites shard_buf; the previous boundary's
            # collectives must have finished reading it (SP issues those DMAs)
            nc.sync.wait_ge(cc_sem, NSEG * (li + 1))
        with tile.TileContext(nc) as tc:
            with (
                tc.tile_pool(name="gp", bufs=6) as gp,
                tc.tile_pool(name="mp", bufs=2) as mp,
                tc.tile_pool(name="agp", bufs=3) as agp,
                tc.tile_pool(name="a33", bufs=2) as a33,
                tc.tile_pool(name="sp", bufs=2) as sp,
                tc.tile_pool(name="const2", bufs=1) as cst_p,
                tc.tile_pool(name="pt", bufs=2, space="PSUM") as pt,
                tc.tile_pool(name="pp", bufs=1, space="PSUM") as pp,
                tc.tile_pool(name="pg", bufs=1, space="PSUM") as pg,
                tc.tile_pool(name="pj", bufs=2, space="PSUM") as pj,
            ):
                cst = {}
                ident32h = cst_p.tile([H, H], F16)
                cst["ident32h"] = ident32h
                make_identity(nc, cst["ident32h"][:])
                ident128 = cst_p.tile([128, 128], F16)
                make_identity(nc, ident128[:])
                zero32 = cst_p.tile([128, H], F16)
                nc.vector.memset(zero32[:], 0.0)
                wnn_t = cst_p.tile([H + 1, H], F16)
                nc.sync.dma_start(
                    out=wnn_t[:], in_=wnn_d[li * (H + 1) : (li + 1) * (H + 1), :]
                )
                wih_t = cst_p.tile([H + 1, 3 * H], F16)
                nc.sync.dma_start(out=wih_t[:], in_=wih_d[:])
                whh_t = cst_p.tile([H + 1, 3 * H], F16)
                nc.sync.dma_start(out=whh_t[:], in_=whh_d[:])
                if last:
                    wout_t = cst_p.tile([H + 1, NCLS], F16)
                    nc.sync.dma_start(out=wout_t[:], in_=wout_d[:])
                    npan = 4 * (b1 - b0)
                    lg_sb = cst_p.tile([128, NCLS * npan], F32)

                table = tables[li]
                for b in range(b0, b1):
                    cols = slice(512 * b, 512 * (b + 1))
                    agg33 = a33.tile([H, 512], F16)
                    tpp = pt.tile([H, 4, 128], F16)
                    for pjj in range(4):
                        p = 4 * b + pjj
                        K = int(K_uni[p])
                        pc = slice(128 * pjj, 128 * (pjj + 1))
                        if K == 0:
                            nc.tensor.transpose(
                                out=tpp[:, pjj, :], in_=zero32[:], identity=ident128[:]
                            )
                            continue
                        gt = gp.tile([128, K, 128], F16)
                        for a in range(-(-K // KCH)):
                            kk = min(KCH, K - KCH * a)
                            ic = int(col0i[p]) + 64 * a
                            nc.gpsimd.dma_gather(
                                out_ap=gt[:, KCH * a : KCH * a + kk, :],
                                in_ap=table[:],
                                idxs_ap=idx_sb[:, ic : ic + 8 * kk],
                                num_idxs=128 * kk,
                                num_idxs_reg=128 * kk,
                                elem_size=128,
                                queue_num=call_q[0] % 4,
                            )
                            call_q[0] += 1
                        # msg[q, f, k, j] = gt[q, k, f, j] * w4[q, k, j]
                        wc = int(col0w[p])
                        msg = mp.tile([128, H, K, 4], F16)
                        nc.vector.tensor_tensor(
                            out=msg[:],
                            in0=gt[:].rearrange("p k (f j) -> p f k j", j=4),
                            in1=w4_sb[:, wc : wc + 4 * K]
                            .rearrange("p (k j) -> p k j", j=4)[:, None, :, :]
                            .to_broadcast([128, H, K, 4]),
                            op=ALU.mult,
                        )
                        agg16 = agp.tile([128, H], F16)
                        nc.vector.tensor_reduce(
                            out=agg16[:],
                            in_=msg[:].rearrange("p f k j -> p f (k j)"),
                            axis=mybir.AxisListType.X,
                            op=ALU.add,
                        )
                        nc.tensor.transpose(
                            out=tpp[:, pjj, :], in_=agg16[:], identity=ident128[:]
                        )
                    nc.scalar.activation(
                        agg33[:], tpp[:].rearrange("p a b -> p (a b)"), AF.Copy
                    )

                    # ---- node phase (feature-major f16, biases via act bias) ----
                    ps1 = pp.tile([H, 512], F32)
                    nc.tensor.matmul(out=ps1[:], lhsT=wnn_t[0:H, :], rhs=agg33[:], start=True, stop=True)
                    oi = a33.tile([H, 512], F16)
                    nc.scalar.activation(
                        oi[:], ps1[:], AF.Identity, bias=bv_sb[0:H, 3 + li : 4 + li]
                    )
                    xfb = xf_cur[:, cols]
                    # r/z gate pre-activations summed in PSUM across both matmuls
                    psrz = pg.tile([2 * H, 512], F32)
                    nc.tensor.matmul(out=psrz[:], lhsT=wih_t[0:H, 0 : 2 * H], rhs=oi[:], start=True, stop=False)
                    nc.tensor.matmul(out=psrz[:], lhsT=whh_t[0:H, 0 : 2 * H], rhs=xfb, start=False, stop=True)
                    psni = pp.tile([H, 512], F32)
                    nc.tensor.matmul(out=psni[:], lhsT=wih_t[0:H, 2 * H : 3 * H], rhs=oi[:], start=True, stop=True)
                    psnh = pp.tile([H, 512], F32)
                    nc.tensor.matmul(out=psnh[:], lhsT=whh_t[0:H, 2 * H : 3 * H], rhs=xfb, start=True, stop=True)

                    r_t = sp.tile([H, 512], F32)
                    nc.scalar.activation(
                        r_t[:], psrz[0:H, :], AF.Sigmoid, bias=bv_sb[0:H, 0:1]
                    )
                    z_t = sp.tile([H, 512], F32)
                    nc.scalar.activation(
                        z_t[:], psrz[H : 2 * H, :], AF.Sigmoid, bias=bv_sb[H : 2 * H, 0:1]
                    )
                    t0 = sp.tile([H, 512], F32)
                    nc.scalar.activation(
                        t0[:], psnh[:], AF.Identity, bias=bv_sb[0:H, 2:3]
                    )
                    t1 = sp.tile([H, 512], F32)
                    nc.vector.tensor_mul(out=t1[:], in0=r_t[:], in1=t0[:])
                    nc.vector.tensor_add(out=t1[:], in0=t1[:], in1=psni[:])
                    n_t = sp.tile([H, 512], F32)
                    nc.scalar.activation(
                        n_t[:], t1[:], AF.Tanh, bias=bv_sb[0:H, 1:2]
                    )
                    # ho = n*(1-z) + (z+fuse)*xf
                    zf = sp.tile([H, 512], F32)
                    nc.scalar.activation(
                        zf[:], z_t[:], AF.Identity, bias=bv_sb[0:H, 5 + li : 6 + li]
                    )
                    a_t = sp.tile([H, 512], F32)
                    nc.vector.tensor_mul(out=a_t[:], in0=zf[:], in1=xfb)
                    c_t = sp.tile([H, 512], F32)
                    nc.vector.tensor_mul(out=c_t[:], in0=n_t[:], in1=z_t[:])
                    nc.vector.tensor_sub(out=c_t[:], in0=n_t[:], in1=c_t[:])
                    if not last:
                        nc.vector.tensor_add(
                            out=xf_nxt[:, cols], in0=c_t[:], in1=a_t[:]
                        )
                        build_rows(tc, (cst, pj, sp), xf_nxt, b)
                        for sg in range(NSEG - 1):
                            if b == sum(SEG_B[: sg + 1]) - 1:
                                emit_seg_coll(li + 1, sg)
                    else:
                        ho33 = a33.tile([H + 1, 512], F16)
                        nc.vector.memset(ho33[H : H + 1, :], 1.0)
                        nc.vector.tensor_add(out=ho33[0:H, :], in0=c_t[:], in1=a_t[:])
                        lps = pj.tile([128, 4 * NCLS], F32)
                        for j in range(4):
                            nc.tensor.matmul(
                                out=lps[:, NCLS * j : NCLS * (j + 1)],
                                lhsT=ho33[:, 128 * j : 128 * (j + 1)],
                                rhs=wout_t[:], start=True, stop=True,
                            )
                        nc.vector.tensor_copy(
                            out=lg_sb[:, NCLS * 4 * (b - b0) : NCLS * 4 * (b - b0 + 1)],
                            in_=lps[:],
                        )

                if not last and b1 == batches:
                    emit_seg_coll(li + 1, NSEG - 1)
                if last:
                    # log_softmax over the 2 classes, node-major [128, npan, 2]
                    lg = lg_sb[:].rearrange("p (n c) -> p n c", c=NCLS)
                    mx = sp.tile([128, npan], F32)
                    nc.vector.tensor_reduce(
                        out=mx[:], in_=lg, axis=mybir.AxisListType.X, op=ALU.max
                    )
                    df = sp.tile([128, npan, NCLS], F32)
                    nc.vector.tensor_tensor(
                        out=df[:],
                        in0=lg,
                        in1=mx[:, :, None].to_broadcast([128, npan, NCLS]),
                        op=ALU.subtract,
                    )
                    ex = sp.tile([128, npan, NCLS], F32)
                    nc.scalar.activation(ex[:], df[:], AF.Exp)
                    sm = sp.tile([128, npan], F32)
                    nc.vector.tensor_reduce(
                        out=sm[:], in_=ex[:], axis=mybir.AxisListType.X, op=ALU.add
                    )
                    nc.scalar.activation(sm[:], sm[:], AF.Ln)
                    ou = sp.tile([128, npan, NCLS], F32)
                    nc.vector.tensor_tensor(
                        out=ou[:],
                        in0=df[:],
                        in1=sm[:, :, None].to_broadcast([128, npan, NCLS]),
                        op=ALU.subtract,
                    )
                    nc.sync.dma_start(
                        out=out_d[:, NCLS * 4 * b0 : NCLS * 4 * b1],
                        in_=ou[:].rearrange("p n c -> p (n c)"),
                    )

    build_layer(0, 0, batches)
    nc.gpsimd.wait_ge(cc_sem, 2 * NSEG)
    build_layer(1, 0, batches)

    nc.compile()
    _split_multiwaits(nc)
    cc_sem_cm.__exit__(None, None, None)
    lp_cm.__exit__(None, None, None)
    _BUILD_CACHE[key] = nc
    return nc


def _prepare(x, edge_index, edge_weight, W_first, b_first, W_nn, b_nn,
             W_ih, b_ih, W_hh, b_hh, fuse_weight, W_out, b_out):
    shard, shard_pad, panels, tabrows = _sizes(N)
    pre = _preprocess(edge_index, edge_weight)
    order = pre["order"]
    fuse = np.asarray(fuse_weight, np.float32)

    nc = _build(pre["K_uni"], fuse)

    x = np.asarray(x, np.float32)
    w1 = _w33(np.asarray(W_first, np.float32).T, b_first)
    wnn = np.concatenate(
        [_w33(np.asarray(W_nn[i], np.float32).T, b_nn[i]) for i in range(LAYERS)], 0
    )
    wih = _w33(np.asarray(W_ih, np.float32).T, b_ih)
    whh = _w33(np.asarray(W_hh, np.float32).T, b_hh)
    wout = _w33(np.asarray(W_out, np.float32).T, b_out)
    bih = np.asarray(b_ih, np.float32)
    bhh = np.asarray(b_hh, np.float32)
    bv = np.zeros((2 * H, 8), np.float32)
    bv[:, 0] = bih[0 : 2 * H] + bhh[0 : 2 * H]       # r/z gate bias
    bv[0:H, 1] = bih[2 * H : 3 * H]                  # n-gate input bias
    bv[0:H, 2] = bhh[2 * H : 3 * H]                  # n-gate hidden bias
    for i in range(LAYERS):
        bv[0:H, 3 + i] = np.asarray(b_nn[i], np.float32)
        bv[0:H, 5 + i] = float(np.asarray(fuse_weight, np.float32)[i])

    in_maps = []
    for c in range(NCORES):
        ids = order[c * shard : (c + 1) * shard]
        xs = np.zeros((H + 1, shard_pad), np.float16)
        xs[0:H, 0:shard] = x[ids].T.astype(np.float16)
        xs[H, :] = 1.0
        in_maps.append(
            {
                "xT": xs,
                "idx": pre["idx_imgs"][c],
                "w4": pre["w4_imgs"][c],
                "w1": w1,
                "wnn": wnn,
                "wih": wih,
                "whh": whh,
                "wout": wout,
                "bv": bv,
            }
        )

    return nc, in_maps, order


def _assemble(order, results):
    shard, shard_pad, panels, tabrows = _sizes(N)
    out = np.zeros((N, NCLS), np.float32)
    for c in range(NCORES):
        R = np.asarray(results[c]["out"])  # [128, 2*panels]
        R = R.reshape(128, panels, NCLS).transpose(1, 0, 2).reshape(-1, NCLS)
        ids = order[c * shard : (c + 1) * shard]
        out[ids] = R[0:shard]
    return out


def kernel(**inputs):
    nc, in_maps, order = _prepare(**inputs)
    res = run_bass_kernel_spmd(nc, in_maps, core_ids=list(range(NCORES)))
    return _assemble(order, res.results)


# revision 36
# speedup vs baseline: 1.0232x; 1.0232x over previous
"""GatedGCN Trainium2 kernel — 8-core SPMD, self-contained.

Strategy (v3)
-------------
dst-shard the graph across 8 NeuronCores. Node features live in DRAM as an
fp16 table with 4 nodes packed per 256B row in FEATURE-MAJOR order
([32f x 4j] per row), so the gathered tile is contiguous in f for the DVE
multiply and the slot-reduction runs as a single XY tensor_reduce per panel
at 16-bit 2x/4x rate. Nodes are relabeled per shard by descending in-degree
and grouped into 128-node panels; each edge occupies a (node, slot) cell, so
the weighted segment-sum is a regular DVE multiply+reduce over the panel (no
scatter). Per-slot weights are expanded into 4 j-planes (weight on the src%4
lane, 0 elsewhere) which also performs the 4-way sub-row selection. The
gather stream (SWDGE, 4 queues, 1024-idx calls) is decoupled from consumers
via per-panel tiles so descriptor generation pipelines at ~2.6ns/idx. Layer
hand-off between cores is a single fp16 AllGather. All matmul operands are
fp16 (PSUM accum f32); GRU r/z gate pre-activations are summed in PSUM by
accumulating the Wih and Whh matmuls into one bank. h stays in SBUF
feature-major fp16 between layers (no DRAM roundtrip).
"""
import sys

sys.path.insert(0, "/opt/trn_rl_repo")

import numpy as np

import concourse.bacc as bacc
import concourse.bass as bass
import concourse.mybir as mybir
import concourse.tile as tile
from concourse.bass_utils import run_bass_kernel_spmd
from concourse.masks import make_identity

N = 100000
E = 1600000
H = 32
NCLS = 2
LAYERS = 2
NCORES = 8
KCH = 8  # idxs per dma_gather call = 128*KCH (SWDGE ring cap ~65-72 descs/dma)

F32 = mybir.dt.float32
F16 = mybir.dt.float16
I16 = mybir.dt.int16
AF = mybir.ActivationFunctionType
ALU = mybir.AluOpType


def _split_multiwaits(nc, max_waits=1):
    """This walrus build rejects >1 sync-wait per instruction; split extras
    onto same-engine InstNoOp predecessors (semantically identical). Also
    collectives only carry one sync update: keep the cc_sem completion update
    on the instruction and move Tile's engine-clock updates to a same-engine
    NoOp successor (fires at issue, which is what the clock tracks)."""
    ctr = 0
    for fn in nc.m.functions:
        for bb in fn.blocks:
            new_insts = []
            for inst in bb.instructions:
                si = inst.sync_info
                if (
                    isinstance(inst, mybir.InstCollectiveCompute)
                    and si is not None
                    and si.on_update
                    and len(si.on_update) > 1
                ):
                    ups = list(si.on_update)
                    keep = [u for u in ups if "cc_sem" in str(u)]
                    move = [u for u in ups if "cc_sem" not in str(u)]
                    if not keep:
                        keep, move = ups[:1], ups[1:]
                    inst.sync_info = mybir.SyncInfo(
                        on_wait=list(si.on_wait) if si.on_wait else [],
                        on_update=keep[:1],
                    )
                    ctr += 1
                    nop = mybir.InstNoOp(name=f"USPLIT-{ctr}", engine=inst.engine)
                    nop.sync_info = mybir.SyncInfo(
                        on_wait=[], on_update=move + keep[1:]
                    )
                    nc.register_instruction(nop, overwrite=True)
                    new_insts.append(inst)
                    new_insts.append(nop)
                    continue
                waits = list(si.on_wait) if si is not None and si.on_wait else []
                if len(waits) > max_waits:
                    head, tail = waits[:-max_waits], waits[-max_waits:]
                    for i in range(0, len(head), max_waits):
                        ctr += 1
                        nop = mybir.InstNoOp(name=f"WSPLIT-{ctr}", engine=inst.engine)
                        nop.sync_info = mybir.SyncInfo(
                            on_wait=head[i : i + max_waits], on_update=[]
                        )
                        nc.register_instruction(nop, overwrite=True)
                        new_insts.append(nop)
                    inst.sync_info = mybir.SyncInfo(
                        on_wait=tail,
                        on_update=list(si.on_update) if si.on_update else [],
                    )
                new_insts.append(inst)
            bb.instructions[:] = new_insts


def _seg_batches(batches):
    """Collective segment sizes (in batches): large head, small tail so the
    final boundary chain is short."""
    return [batches // 2 + 1, batches - batches // 2 - 1]


def _sizes(n):
    shard = n // NCORES
    shard_pad = -(-shard // 512) * 512
    panels = shard_pad // 128
    tabrows = NCORES * shard_pad // 4
    return shard, shard_pad, panels, tabrows


def _preprocess(edge_index, edge_weight):
    shard, shard_pad, panels, tabrows = _sizes(N)
    src_ = np.asarray(edge_index[0], dtype=np.int64)
    dst = np.asarray(edge_index[1], dtype=np.int64)
    src = src_
    w = np.asarray(edge_weight, dtype=np.float32)

    deg = np.bincount(dst, minlength=N)
    shards = np.arange(N) // shard
    order = np.lexsort((np.arange(N), -deg, shards))  # old ids by (shard, -deg)
    new_of_old = np.empty(N, dtype=np.int64)
    pos = np.arange(N)
    c_of_pos = pos // shard
    r_of_pos = pos - c_of_pos * shard
    new_of_old[order] = c_of_pos * shard_pad + r_of_pos

    s_new = new_of_old[src]
    d_new = new_of_old[dst]
    core = d_new // shard_pad
    r = d_new % shard_pad
    # slot index k per edge: occurrence number among edges sharing the dst
    eorder = np.argsort(d_new, kind="stable")
    ds = d_new[eorder]
    starts = np.r_[0, np.nonzero(np.diff(ds))[0] + 1]
    counts = np.diff(np.r_[starts, len(ds)])
    k_sorted = np.arange(len(ds)) - np.repeat(starts, counts)
    k = np.empty(src_.size, dtype=np.int64)
    k[eorder] = k_sorted

    # per-core per-panel K, unified across cores (SPMD: one program)
    deg_new = np.zeros(NCORES * shard_pad, dtype=np.int64)
    deg_new[new_of_old] = deg
    K_uni = np.zeros(panels, dtype=np.int64)
    for c in range(NCORES):
        base = c * shard_pad
        firsts = deg_new[base : base + shard_pad : 128]  # max of each panel
        K_uni = np.maximum(K_uni, firsts)
    K_uni = K_uni.astype(np.int64)
    sumK = int(K_uni.sum())
    col0 = np.zeros(panels + 1, dtype=np.int64)
    col0[1:] = np.cumsum(128 * K_uni)
    slots_total = int(col0[-1])

    p_of_edge = r // 128
    q_of_edge = r % 128
    slotpos = col0[p_of_edge] + k * 128 + q_of_edge

    # segment-major table rows: the AllGather runs in segments, so the table
    # is [8 x seg0 rows][8 x seg1 rows]... with per-core blocks contiguous
    seg_rows = [b * 128 for b in _seg_batches(shard_pad // 512)]
    s_core = s_new // shard_pad
    s_row = (s_new % shard_pad) >> 2
    row_glob = np.zeros_like(s_row)
    r0 = 0
    t0 = 0
    for sr in seg_rows:
        m2 = (s_row >= r0) & (s_row < r0 + sr)
        row_glob[m2] = t0 + s_core[m2] * sr + (s_row[m2] - r0)
        r0 += sr
        t0 += NCORES * sr
    row_glob = row_glob.astype(np.int16)
    idx_imgs = np.zeros((NCORES, 128, 8 * sumK), dtype=np.int16)
    w4_imgs = np.zeros((NCORES, 128, 4 * sumK), dtype=np.float16)
    for c in range(NCORES):
        m = core == c
        ia = np.zeros(slots_total, dtype=np.int16)
        wa = np.zeros(slots_total * 4, dtype=np.float16)
        ia[slotpos[m]] = row_glob[m]
        wa[slotpos[m] * 4 + (s_new[m] & 3)] = w[m].astype(np.float16)
        icols = 0
        wcols = 0
        for p in range(panels):
            K = int(K_uni[p])
            if K == 0:
                continue
            a, b = int(col0[p]), int(col0[p + 1])
            blk = ia[a:b].reshape(K * 8, 16).T  # [16, 8K]
            idx_imgs[c, :, icols : icols + 8 * K] = np.tile(blk, (8, 1))
            wb = wa[4 * a : 4 * b].reshape(K, 128, 4)
            w4_imgs[c, :, wcols : wcols + 4 * K] = wb.transpose(1, 0, 2).reshape(
                128, 4 * K
            )
            icols += 8 * K
            wcols += 4 * K
    return {
        "order": order,
        "K_uni": K_uni,
        "idx_imgs": idx_imgs,
        "w4_imgs": w4_imgs,
        "sumK": sumK,
    }


def _w33(WT, b):
    """[K_in, K_out] weight^T stacked with bias row -> [K_in+1, K_out] f16."""
    return np.concatenate(
        [np.asarray(WT, np.float32), np.asarray(b, np.float32)[None, :]], axis=0
    ).astype(np.float16)


_BUILD_CACHE = {}


def _build(K_uni, fuse):
    key = (tuple(int(x) for x in K_uni), tuple(float(x) for x in fuse))
    if key in _BUILD_CACHE:
        return _BUILD_CACHE[key]

    shard, shard_pad, panels, tabrows = _sizes(N)
    sumK = int(np.sum(K_uni))
    batches = panels // 4

    nc = bacc.Bacc(
        "TRN2",
        target_bir_lowering=False,
        debug=False,
        num_devices=NCORES,
        num_swdge_queues=4,
        dynamic_dma_scratch_size=8192,
    )
    lp_cm = nc.allow_low_precision(reason="f16 msg accumulate, tol 2e-2")
    lp_cm.__enter__()
    xT = nc.dram_tensor("xT", [H + 1, shard_pad], F16, kind="ExternalInput").ap()
    idx_d = nc.dram_tensor("idx", [128, 8 * sumK], I16, kind="ExternalInput").ap()
    w4_d = nc.dram_tensor("w4", [128, 4 * sumK], F16, kind="ExternalInput").ap()
    w1_d = nc.dram_tensor("w1", [H + 1, H], F16, kind="ExternalInput").ap()
    wnn_d = nc.dram_tensor("wnn", [LAYERS * (H + 1), H], F16, kind="ExternalInput").ap()
    wih_d = nc.dram_tensor("wih", [H + 1, 3 * H], F16, kind="ExternalInput").ap()
    whh_d = nc.dram_tensor("whh", [H + 1, 3 * H], F16, kind="ExternalInput").ap()
    wout_d = nc.dram_tensor("wout", [H + 1, NCLS], F16, kind="ExternalInput").ap()
    bv_d = nc.dram_tensor("bv", [2 * H, 8], F32, kind="ExternalInput").ap()
    out_d = nc.dram_tensor("out", [128, NCLS * panels], F32, kind="ExternalOutput").ap()

    shard_buf = nc.dram_tensor("shard_buf", [shard_pad // 4, 128], F16).ap()
    tables = [
        nc.dram_tensor(f"table{i}", [tabrows, 128], F16, addr_space="Shared").ap()
        for i in range(LAYERS)
    ]

    # persistent SBUF: gather indices / weight planes / feature-major h (f16,
    # [33, shard_pad] with a ones row at partition 32 for bias matmuls)
    idx_sb = nc.alloc_sbuf_tensor("idx_sb", [128, 8 * sumK], I16).ap()
    bv_sb = nc.alloc_sbuf_tensor("bv_sb", [2 * H, 8], F32).ap()
    w4_sb = nc.alloc_sbuf_tensor("w4_sb", [128, 4 * sumK], F16).ap()
    xf = [
        nc.alloc_sbuf_tensor(f"xf{i}", [H, shard_pad], F16).ap() for i in range(2)
    ]

    cc_sem_cm = nc.semaphore("cc_sem")
    cc_sem = cc_sem_cm.__enter__()

    call_q = [0]
    SEG_B = _seg_batches(batches)          # batches per collective segment
    SEG_R = [b * 128 for b in SEG_B]       # shard_buf rows per segment
    NSEG = len(SEG_B)
    rg = [list(range(NCORES))]

    def emit_seg_coll(li, seg):
        """AllGather one shard_buf segment into the segment-major table."""
        tab_flat = tables[li].rearrange("a b -> (a b)")
        r0 = sum(SEG_R[:seg])
        r1 = r0 + SEG_R[seg]
        t0 = NCORES * r0 * 128
        t1 = NCORES * r1 * 128
        nc.gpsimd.collective_compute(
            "AllGather", ALU.bypass, replica_groups=rg,
            ins=[shard_buf[r0:r1, :].rearrange("a b -> (a b)")],
            outs=[tab_flat[t0:t1]],
        ).then_inc(cc_sem, 1)

    def build_rows(tc, pools, src33, b):
        """Emit table rows ([32f x 4j] per 4-node row) for batch b from the
        feature-major [33, shard_pad] f16 tensor src33, then DMA to shard_buf."""
        cst, pj, sp = pools
        cols = slice(512 * b, 512 * (b + 1))
        jt = pj.tile([128, 4, H], F16)
        for j in range(4):
            nc.tensor.transpose(
                out=jt[:, j, :],
                in_=src33[:, cols][:, j::4],
                identity=cst["ident32h"][:],
            )
        hfp = sp.tile([128, H, 4], F16)
        # reorder (j, f) -> (f, j): iterate out in (j, f) order to match jt
        nc.vector.tensor_copy(
            out=hfp[:].rearrange("p f j -> p j f"),
            in_=jt[:],
        )
        nc.sync.dma_start(
            out=shard_buf[128 * b : 128 * (b + 1), :],
            in_=hfp[:].rearrange("p a b -> p (a b)"),
        )

    # ---------------- TC1: h1 = relu(x @ W1 + b1) for own shard ----------
    with tile.TileContext(nc) as tc:
        with (
            tc.tile_pool(name="cp", bufs=2) as cp,
            tc.tile_pool(name="sp", bufs=2) as sp,
            tc.tile_pool(name="const1", bufs=1) as cst_p,
            tc.tile_pool(name="pp", bufs=2, space="PSUM") as pp,
            tc.tile_pool(name="pj", bufs=2, space="PSUM") as pj,
        ):
            cst = {}
            ident32h = cst_p.tile([H, H], F16)
            cst["ident32h"] = ident32h
            make_identity(nc, cst["ident32h"][:])
            w1t = cst_p.tile([H + 1, H], F16)
            nc.sync.dma_start(out=w1t[:], in_=w1_d[:])
            nc.scalar.dma_start(out=idx_sb[:], in_=idx_d[:])
            nc.sync.dma_start(out=bv_sb[:], in_=bv_d[:])
            nc.scalar.dma_start(out=w4_sb[:], in_=w4_d[:])
            for b in range(batches):
                cols = slice(512 * b, 512 * (b + 1))
                xt = cp.tile([H + 1, 512], F16)
                nc.sync.dma_start(out=xt[:], in_=xT[:, cols])
                ps = pp.tile([H, 512], F32)
                nc.tensor.matmul(out=ps[:], lhsT=w1t[:], rhs=xt[:], start=True, stop=True)
                nc.scalar.activation(xf[0][:, cols], ps[:], AF.Relu)
                build_rows(tc, (cst, pj, sp), xf[0], b)
                for sg in range(NSEG - 1):
                    if b == sum(SEG_B[: sg + 1]) - 1:
                        emit_seg_coll(0, sg)
            emit_seg_coll(0, NSEG - 1)

    nc.gpsimd.wait_ge(cc_sem, NSEG)

    # ---------------- layers ----------------
    def build_layer(li, b0, b1):
        last = li == LAYERS - 1
        col0i = np.zeros(panels + 1, dtype=np.int64)
        col0i[1:] = np.cumsum(8 * K_uni)
        col0w = np.zeros(panels + 1, dtype=np.int64)
        col0w[1:] = np.cumsum(4 * K_uni)
        xf_cur, xf_nxt = xf[li % 2], xf[(li + 1) % 2]
        if not last:
            # build_rows below overwrites shard_buf; the previous boundary's
            # collectives must have finished reading it (SP issues those DMAs)
            nc.sync.wait_ge(cc_sem, NSEG * (li + 1))
        with tile.TileContext(nc) as tc:
            with (
                tc.tile_pool(name="gp", bufs=5) as gp,
                tc.tile_pool(name="mp", bufs=3) as mp,
                tc.tile_pool(name="agp", bufs=3) as agp,
                tc.tile_pool(name="a33", bufs=2) as a33,
                tc.tile_pool(name="sp", bufs=2) as sp,
                tc.tile_pool(name="const2", bufs=1) as cst_p,
                tc.tile_pool(name="pt", bufs=2, space="PSUM") as pt,
                tc.tile_pool(name="pp", bufs=1, space="PSUM") as pp,
                tc.tile_pool(name="pg", bufs=1, space="PSUM") as pg,
                tc.tile_pool(name="pj", bufs=2, space="PSUM") as pj,
            ):
                cst = {}
                ident32h = cst_p.tile([H, H], F16)
                cst["ident32h"] = ident32h
                make_identity(nc, cst["ident32h"][:])
                ident128 = cst_p.tile([128, 128], F16)
                make_identity(nc, ident128[:])
                zero32 = cst_p.tile([128, H], F16)
                nc.vector.memset(zero32[:], 0.0)
                wnn_t = cst_p.tile([H + 1, H], F16)
                nc.sync.dma_start(
                    out=wnn_t[:], in_=wnn_d[li * (H + 1) : (li + 1) * (H + 1), :]
                )
                wih_t = cst_p.tile([H + 1, 3 * H], F16)
                nc.sync.dma_start(out=wih_t[:], in_=wih_d[:])
                whh_t = cst_p.tile([H + 1, 3 * H], F16)
                nc.sync.dma_start(out=whh_t[:], in_=whh_d[:])
                if last:
                    wout_t = cst_p.tile([H + 1, NCLS], F16)
                    nc.sync.dma_start(out=wout_t[:], in_=wout_d[:])
                    npan = 4 * (b1 - b0)
                    lg_sb = cst_p.tile([128, NCLS * npan], F32)

                table = tables[li]
                for b in range(b0, b1):
                    cols = slice(512 * b, 512 * (b + 1))
                    agg33 = a33.tile([H, 512], F16)
                    tpp = pt.tile([H, 4, 128], F16)
                    for pjj in range(4):
                        p = 4 * b + pjj
                        K = int(K_uni[p])
                        pc = slice(128 * pjj, 128 * (pjj + 1))
                        if K == 0:
                            nc.tensor.transpose(
                                out=tpp[:, pjj, :], in_=zero32[:], identity=ident128[:]
                            )
                            continue
                        gt = gp.tile([128, K, 128], F16)
                        for a in range(-(-K // KCH)):
                            kk = min(KCH, K - KCH * a)
                            ic = int(col0i[p]) + 64 * a
                            nc.gpsimd.dma_gather(
                                out_ap=gt[:, KCH * a : KCH * a + kk, :],
                                in_ap=table[:],
                                idxs_ap=idx_sb[:, ic : ic + 8 * kk],
                                num_idxs=128 * kk,
                                num_idxs_reg=128 * kk,
                                elem_size=128,
                                queue_num=call_q[0] % 4,
                            )
                            call_q[0] += 1
                        # msg[q, f, k, j] = gt[q, k, f, j] * w4[q, k, j]
                        wc = int(col0w[p])
                        msg = mp.tile([128, H, K, 4], F16)
                        nc.vector.tensor_tensor(
                            out=msg[:],
                            in0=gt[:].rearrange("p k (f j) -> p f k j", j=4),
                            in1=w4_sb[:, wc : wc + 4 * K]
                            .rearrange("p (k j) -> p k j", j=4)[:, None, :, :]
                            .to_broadcast([128, H, K, 4]),
                            op=ALU.mult,
                        )
                        agg16 = agp.tile([128, H], F16)
                        nc.vector.tensor_reduce(
                            out=agg16[:],
                            in_=msg[:].rearrange("p f k j -> p f (k j)"),
                            axis=mybir.AxisListType.X,
                            op=ALU.add,
                        )
                        nc.tensor.transpose(
                            out=tpp[:, pjj, :], in_=agg16[:], identity=ident128[:]
                        )
                    nc.scalar.activation(
                        agg33[:], tpp[:].rearrange("p a b -> p (a b)"), AF.Copy
                    )

                    # ---- node phase (feature-major f16, biases via act bias) ----
                    ps1 = pp.tile([H, 512], F32)
                    nc.tensor.matmul(out=ps1[:], lhsT=wnn_t[0:H, :], rhs=agg33[:], start=True, stop=True)
                    oi = a33.tile([H, 512], F16)
                    nc.scalar.activation(
                        oi[:], ps1[:], AF.Identity, bias=bv_sb[0:H, 3 + li : 4 + li]
                    )
                    xfb = xf_cur[:, cols]
                    # r/z gate pre-activations summed in PSUM across both matmuls
                    psrz = pg.tile([2 * H, 512], F32)
                    nc.tensor.matmul(out=psrz[:], lhsT=wih_t[0:H, 0 : 2 * H], rhs=oi[:], start=True, stop=False)
                    nc.tensor.matmul(out=psrz[:], lhsT=whh_t[0:H, 0 : 2 * H], rhs=xfb, start=False, stop=True)
                    psni = pp.tile([H, 512], F32)
                    nc.tensor.matmul(out=psni[:], lhsT=wih_t[0:H, 2 * H : 3 * H], rhs=oi[:], start=True, stop=True)
                    psnh = pp.tile([H, 512], F32)
                    nc.tensor.matmul(out=psnh[:], lhsT=whh_t[0:H, 2 * H : 3 * H], rhs=xfb, start=True, stop=True)

                    r_t = sp.tile([H, 512], F32)
                    nc.scalar.activation(
                        r_t[:], psrz[0:H, :], AF.Sigmoid, bias=bv_sb[0:H, 0:1]
                    )
                    z_t = sp.tile([H, 512], F32)
                    nc.scalar.activation(
                        z_t[:], psrz[H : 2 * H, :], AF.Sigmoid, bias=bv_sb[H : 2 * H, 0:1]
                    )
                    t0 = sp.tile([H, 512], F32)
                    nc.scalar.activation(
                        t0[:], psnh[:], AF.Identity, bias=bv_sb[0:H, 2:3]
                    )
                    t1 = sp.tile([H, 512], F32)
                    nc.vector.tensor_mul(out=t1[:], in0=r_t[:], in1=t0[:])
                    nc.vector.tensor_add(out=t1[:], in0=t1[:], in1=psni[:])
                    n_t = sp.tile([H, 512], F32)
                    nc.scalar.activation(
                        n_t[:], t1[:], AF.Tanh, bias=bv_sb[0:H, 1:2]
                    )
                    # ho = n*(1-z) + (z+fuse)*xf
                    zf = sp.tile([H, 512], F32)
                    nc.scalar.activation(
                        zf[:], z_t[:], AF.Identity, bias=bv_sb[0:H, 5 + li : 6 + li]
                    )
                    a_t = sp.tile([H, 512], F32)
                    nc.vector.tensor_mul(out=a_t[:], in0=zf[:], in1=xfb)
                    c_t = sp.tile([H, 512], F32)
                    nc.vector.tensor_mul(out=c_t[:], in0=n_t[:], in1=z_t[:])
                    nc.vector.tensor_sub(out=c_t[:], in0=n_t[:], in1=c_t[:])
                    if not last:
                        nc.vector.tensor_add(
                            out=xf_nxt[:, cols], in0=c_t[:], in1=a_t[:]
                        )
                        build_rows(tc, (cst, pj, sp), xf_nxt, b)
                        for sg in range(NSEG - 1):
                            if b == sum(SEG_B[: sg + 1]) - 1:
                                emit_seg_coll(li + 1, sg)
                    else:
                        ho33 = a33.tile([H + 1, 512], F16)
                        nc.vector.memset(ho33[H : H + 1, :], 1.0)
                        nc.vector.tensor_add(out=ho33[0:H, :], in0=c_t[:], in1=a_t[:])
                        lps = pj.tile([128, 4 * NCLS], F32)
                        for j in range(4):
                            nc.tensor.matmul(
                                out=lps[:, NCLS * j : NCLS * (j + 1)],
                                lhsT=ho33[:, 128 * j : 128 * (j + 1)],
                                rhs=wout_t[:], start=True, stop=True,
                            )
                        nc.vector.tensor_copy(
                            out=lg_sb[:, NCLS * 4 * (b - b0) : NCLS * 4 * (b - b0 + 1)],
                            in_=lps[:],
                        )

                if not last and b1 == batches:
                    emit_seg_coll(li + 1, NSEG - 1)
                if last:
                    # log_softmax over the 2 classes, node-major [128, npan, 2]
                    lg = lg_sb[:].rearrange("p (n c) -> p n c", c=NCLS)
                    mx = sp.tile([128, npan], F32)
                    nc.vector.tensor_reduce(
                        out=mx[:], in_=lg, axis=mybir.AxisListType.X, op=ALU.max
                    )
                    df = sp.tile([128, npan, NCLS], F32)
                    nc.vector.tensor_tensor(
                        out=df[:],
                        in0=lg,
                        in1=mx[:, :, None].to_broadcast([128, npan, NCLS]),
                        op=ALU.subtract,
                    )
                    ex = sp.tile([128, npan, NCLS], F32)
                    nc.scalar.activation(ex[:], df[:], AF.Exp)
                    sm = sp.tile([128, npan], F32)
                    nc.vector.tensor_reduce(
                        out=sm[:], in_=ex[:], axis=mybir.AxisListType.X, op=ALU.add
                    )
                    nc.scalar.activation(sm[:], sm[:], AF.Ln)
                    ou = sp.tile([128, npan, NCLS], F32)
                    nc.vector.tensor_tensor(
                        out=ou[:],
                        in0=df[:],
                        in1=sm[:, :, None].to_broadcast([128, npan, NCLS]),
                        op=ALU.subtract,
                    )
                    nc.sync.dma_start(
                        out=out_d[:, NCLS * 4 * b0 : NCLS * 4 * b1],
                        in_=ou[:].rearrange("p n c -> p (n c)"),
                    )

    build_layer(0, 0, batches)
    nc.gpsimd.wait_ge(cc_sem, 2 * NSEG)
    build_layer(1, 0, batches)

    nc.compile()
    _split_multiwaits(nc)
    cc_sem_cm.__exit__(None, None, None)
    lp_cm.__exit__(None, None, None)
    _BUILD_CACHE[key] = nc
    return nc


def _prepare(x, edge_index, edge_weight, W_first, b_first, W_nn, b_nn,
             W_ih, b_ih, W_hh, b_hh, fuse_weight, W_out, b_out):
    shard, shard_pad, panels, tabrows = _sizes(N)
    pre = _preprocess(edge_index, edge_weight)
    order = pre["order"]
    fuse = np.asarray(fuse_weight, np.float32)

    nc = _build(pre["K_uni"], fuse)

    x = np.asarray(x, np.float32)
    w1 = _w33(np.asarray(W_first, np.float32).T, b_first)
    wnn = np.concatenate(
        [_w33(np.asarray(W_nn[i], np.float32).T, b_nn[i]) for i in range(LAYERS)], 0
    )
    wih = _w33(np.asarray(W_ih, np.float32).T, b_ih)
    whh = _w33(np.asarray(W_hh, np.float32).T, b_hh)
    wout = _w33(np.asarray(W_out, np.float32).T, b_out)
    bih = np.asarray(b_ih, np.float32)
    bhh = np.asarray(b_hh, np.float32)
    bv = np.zeros((2 * H, 8), np.float32)
    bv[:, 0] = bih[0 : 2 * H] + bhh[0 : 2 * H]       # r/z gate bias
    bv[0:H, 1] = bih[2 * H : 3 * H]                  # n-gate input bias
    bv[0:H, 2] = bhh[2 * H : 3 * H]                  # n-gate hidden bias
    for i in range(LAYERS):
        bv[0:H, 3 + i] = np.asarray(b_nn[i], np.float32)
        bv[0:H, 5 + i] = float(np.asarray(fuse_weight, np.float32)[i])

    in_maps = []
    for c in range(NCORES):
        ids = order[c * shard : (c + 1) * shard]
        xs = np.zeros((H + 1, shard_pad), np.float16)
        xs[0:H, 0:shard] = x[ids].T.astype(np.float16)
        xs[H, :] = 1.0
        in_maps.append(
            {
                "xT": xs,
                "idx": pre["idx_imgs"][c],
                "w4": pre["w4_imgs"][c],
                "w1": w1,
                "wnn": wnn,
                "wih": wih,
                "whh": whh,
                "wout": wout,
                "bv": bv,
            }
        )

    return nc, in_maps, order


def _assemble(order, results):
    shard, shard_pad, panels, tabrows = _sizes(N)
    out = np.zeros((N, NCLS), np.float32)
    for c in range(NCORES):
        R = np.asarray(results[c]["out"])  # [128, 2*panels]
        R = R.reshape(128, panels, NCLS).transpose(1, 0, 2).reshape(-1, NCLS)
        ids = order[c * shard : (c + 1) * shard]
        out[ids] = R[0:shard]
    return out


def kernel(**inputs):
    nc, in_maps, order = _prepare(**inputs)
    res = run_bass_kernel_spmd(nc, in_maps, core_ids=list(range(NCORES)))
    return _assemble(order, res.results)
